# revision 32
# baseline (speedup 1.0000x reference)
"""Trainium2 Bass kernel for the attention-context module:

    query  = dec @ W.T                      # [B, H]
    scores = einsum('bsh,bh->bs', enc, q)   # raw scores (no softmax!)
    ctx    = einsum('bs,bsh->bh', scores, enc)[:, None, :]

Data-parallel over the batch dim: 8 NeuronCores x 4 batches each.  Per core
the kernel streams its 32 MiB enc shard through SBUF exactly once on the SP
HWDGE ring (one queue already fans out to all 16 DMA engines at ~390 GB/s
while active; a second ring or SWDGE splitting only slows it down), in
[128s x 8*512h] supertiles.

Default mode "stt" keeps every engine under the ~91 us DMA stream time:
  - 4 of 8 tiles/supertile: ONE fused DVE scalar_tensor_tensor writes
    tmp = E*q (f32r, feeds the matmul) AND the scores column via accum_out
    — mul+reduce in a single 512-elem/partition pass.
  - the other 4 tiles: one batched DVE tensor_mul (cheaper per tile than
    STT) + ACT Copy-activation accum_out reduces (ACT is otherwise idle).
  - PE: y[1,512] += scores_j.T @ tmp_j as f32r matmuls (1 cyc/row);
    per batch ctx = y * (1/q) recovers the E-weighted sum since tmp = E*q.
    1/q for all batches comes from one reciprocal_approx_fast (no 13us
    DVE table op); absmax/scale ~5e-5, dominated by the f32r rounding.
  - gpsimd (Pool) only broadcasts q rows and issues the tiny SWDGE output
    stores; Pool tensor ops on big tiles measured 2.5x slower than DVE and
    jam the SBUF ports — keep it off the streaming path.

Steady-state pacing details: const tiles are double-buffered
(CONST_BUFS=2) so the next rep's W/dec/ident loads don't stall the FIFO
ring behind the previous rep's epilogue, and the last batch's short
drain-segments (4KB DMA packets, ~2x slower per byte) are emitted only on
the final rep.  The query q = dec @ W.T is computed on-chip via PE
transposes + fp32 matmuls, keeping DVE free.  Fallback modes (env KMODE):
brx = baseline unfused DVE-mul/ACT-reduce split, b32 = exact-fp32 matmul
(absmax/scale ~9e-7), a32 = fp32 with E stationary.
"""

import os
import sys

import numpy as np

for _p in ("/opt/trn_rl_repo", "/root/.axon_site/_ro/trn_rl_repo"):
    if os.path.isdir(_p) and _p not in sys.path:
        sys.path.append(_p)

import concourse.bacc as bacc
import concourse.bass as bass
import concourse.mybir as mybir
import concourse.tile as tile
from concourse.bass_utils import run_bass_kernel_spmd

N_CORES = 8
B, S, H = 32, 4096, 512
B_LOC = B // N_CORES          # 4 batches per core
G = int(os.environ.get("G_OVERRIDE", "8"))  # 128-row s-tiles per supertile
ST_ROWS = 128 * G             # 1024 enc rows per supertile
N_ST = S // ST_ROWS           # 4 supertiles per batch
F32 = mybir.dt.float32
F32R = mybir.dt.float32r

MODE = os.environ.get("KMODE", "stt")
# in stt mode: how many of the 8 per-supertile tiles take the
# DVE-batched-mul + ACT-reduce path (rest: fused DVE STT with accum_out)
STT_ACT_TILES = int(os.environ.get("STT_ACT_TILES", "4"))
# in brx mode: how many of the 8 per-supertile reduces run on DVE (rest ACT)
BRX_DVE_TILES = int(os.environ.get("BRX_DVE_TILES", "2"))
# partition-major supertile DMA: 16KB contiguous per partition (1 descriptor)
DMA_PMAJOR = os.environ.get("DMA_PMAJOR", "1") == "1"
ST_BUFS = int(os.environ.get("ST_BUFS", "4"))
TMP_BUFS = int(os.environ.get("TMP_BUFS", "3"))
# model-probing only: skip an engine's work to find the bottleneck
SKIP = set(os.environ.get("KSKIP", "").split(","))
# offload q_1p PSUM->SBUF copies to ACT instead of DVE
Q1P_ACT = os.environ.get("Q1P_ACT", "1") == "1"
# run the 16 query STT ops on gpsimd instead of DVE
QSTT_GP = os.environ.get("QSTT_GP", "0") == "1"
# alternate supertile DMAs between the two HWDGE rings (SP / ACT)
DMA_ALT = os.environ.get("DMA_ALT", "0") == "1"
# every Nth supertile DMA goes through Pool SWDGE as a second descriptor
# generator (0 = off); the 16 DMA engines are shared either way
DMA_GP_NTH = int(os.environ.get("DMA_GP_NTH", "0"))
# query computation path: "stt" (DVE fused mul+reduce) or "pe" (transpose+matmul)
QMODE = os.environ.get("QMODE", "pe")
# which HWDGE ring carries the small const loads: "sp" or "act"
CONST_RING = os.environ.get("CONST_RING", "sp")
# which engine issues the tiny output stores: "sp" or "gp"
OUT_ENG = os.environ.get("OUT_ENG", "gp")


def build_nc(mode: str = MODE, reps: int = 1) -> bass.Bass:
    assert mode in ("b32", "a32", "br", "brx", "stt")
    nc = bacc.Bacc("TRN2", target_bir_lowering=False, debug=False,
                   num_devices=N_CORES)

    enc = nc.dram_tensor("enc", [B_LOC, S, H], F32, kind="ExternalInput").ap()
    dec = nc.dram_tensor("dec", [B_LOC, H], F32, kind="ExternalInput").ap()
    w = nc.dram_tensor("w", [H, H], F32, kind="ExternalInput").ap()
    ident = nc.dram_tensor("ident", [128, 128], F32, kind="ExternalInput").ap()
    out = nc.dram_tensor("out", [B_LOC, H], F32, kind="ExternalOutput").ap()

    with tile.TileContext(nc) as tc:
        with (
            tc.tile_pool(name="const", bufs=int(os.environ.get("CONST_BUFS", "2"))) as const_pool,
            tc.tile_pool(name="sts", bufs=ST_BUFS) as st_pool,
            tc.tile_pool(name="tmps", bufs=TMP_BUFS) as tmp_pool,
            tc.tile_pool(name="scores", bufs=3) as sc_pool,
            tc.tile_pool(name="qb", bufs=2) as qb_pool,
            tc.tile_pool(name="osb", bufs=int(os.environ.get("OUT_BUFS", "2"))) as out_pool,
            tc.tile_pool(name="psum", bufs=int(os.environ.get("PSUM_BUFS", "2")), space="PSUM") as psum_pool,
            tc.tile_pool(name="psum_acc", bufs=1, space="PSUM") as psum_acc_pool,
            tc.tile_pool(name="psum_q", bufs=2, space="PSUM") as psum_q_pool,
        ):
            for _rep in range(reps):
                # ---- constants / small inputs -------------------------------
                # const loads can ride the ACT HWDGE ring so they land during
                # the previous iteration's enc streaming on the SP ring
                _cdma = nc.scalar if CONST_RING == "act" else nc.sync
                # tiny const loads must not be starved by the 2MB enc stream:
                # pull them to the front of the scheduler's priority heap
                _cprio = -100000 + _rep * 100
                if QMODE != "pe":
                    # dec rows packed on partition 0 (for partition_broadcast);
                    # only the stt query path reads this
                    dec_1p = const_pool.tile([1, B_LOC * H], F32, tag="dec1p")
                    _cdma.dma_start(
                        dec_1p[:], dec.rearrange("b h -> (b h)").unsqueeze(0)
                    ).bass_priority = _cprio

                # W[c*128+p, h] -> w_sb[p, c*512+h]; chunked so the first
                # query STT can start as soon as chunk 0 lands
                w_sb = const_pool.tile([128, 4 * H], F32, tag="w")
                wr = w.rearrange("(c p) h -> p c h", p=128)
                for c in range(4):
                    _cdma.dma_start(
                        w_sb[:, c * H:(c + 1) * H], wr[:, c, :]
                    ).bass_priority = _cprio + 2 + c
                ident_sb = const_pool.tile([128, 128], F32, tag="ident")
                _cdma.dma_start(ident_sb[:], ident).bass_priority = _cprio + 1

                # ---- query: Q[b, o] = sum_h dec[b, h] * W[o, h] -------------
                qcols = const_pool.tile([128, 4 * B_LOC], F32, tag="qcols")
                if QMODE == "pe":
                    # transpose W and dec on the PE, then qcols = WT.T? no:
                    # qcols_c[o, b] = sum_h WT[h, c*128+o] * decT[h, b]
                    dec_sb4 = const_pool.tile([B_LOC, H], F32, tag="dec4")
                    _cdma.dma_start(dec_sb4[:], dec).bass_priority = _cprio
                    dect = const_pool.tile([128, 4 * B_LOC], F32, tag="dect")
                    for hc in range(4):
                        dt_ps = psum_q_pool.tile([128, B_LOC], F32, tag="qsm")
                        nc.tensor.transpose(
                            dt_ps[:], dec_sb4[:, hc * 128:(hc + 1) * 128],
                            ident_sb[0:B_LOC, 0:B_LOC])
                        nc.scalar.copy(
                            dect[:, hc * B_LOC:(hc + 1) * B_LOC], dt_ps[:])
                    wt_sb = const_pool.tile([128, 4 * H], F32, tag="wt")
                    for c in range(4):
                        for hc in range(4):
                            wt_ps = psum_q_pool.tile([128, 128], F32, tag="wtp")
                            nc.tensor.transpose(
                                wt_ps[:],
                                w_sb[:, c * H + hc * 128:c * H + (hc + 1) * 128],
                                ident_sb[:])
                            nc.scalar.copy(
                                wt_sb[:, hc * H + c * 128:hc * H + (c + 1) * 128],
                                wt_ps[:])
                        qc_ps = psum_q_pool.tile([128, B_LOC], F32, tag="qsm")
                        for hc in range(4):
                            nc.tensor.matmul(
                                qc_ps[:],
                                wt_sb[:, hc * H + c * 128:hc * H + (c + 1) * 128],
                                dect[:, hc * B_LOC:(hc + 1) * B_LOC],
                                start=(hc == 0), stop=(hc == 3),
                            )
                        nc.scalar.copy(
                            qcols[:, c * B_LOC:(c + 1) * B_LOC], qc_ps[:])
                else:
                    dec_bc = const_pool.tile([128, B_LOC * H], F32, tag="dec_bc")
                    for b in range(B_LOC):
                        nc.gpsimd.partition_broadcast(
                            dec_bc[:, b * H:(b + 1) * H],
                            dec_1p[:, b * H:(b + 1) * H])

                    # fused multiply + free-dim sum -> Q columns
                    trash_v = const_pool.tile([128, H], F32, tag="trash_v")
                    _stt_eng = nc.gpsimd if QSTT_GP else nc.vector
                    for c in range(4):
                        for b in range(B_LOC):
                            _stt_eng.scalar_tensor_tensor(
                                out=trash_v[:],
                                in0=w_sb[:, c * H:(c + 1) * H],
                                scalar=1.0,
                                in1=dec_bc[:, b * H:(b + 1) * H],
                                op0=mybir.AluOpType.mult,
                                op1=mybir.AluOpType.mult,
                                accum_out=qcols[:, c * B_LOC + b:
                                                c * B_LOC + b + 1],
                            )

                # transpose Q columns into rows on partition 0: q_1p[0, b*H + o]
                q_1p = const_pool.tile([1, B_LOC * H], F32, tag="q1p")
                for b in range(B_LOC):
                    for c in range(4):
                        qt_ps = psum_q_pool.tile([1, 128], F32, tag="qt")
                        col = c * B_LOC + b
                        nc.tensor.transpose(
                            qt_ps[:], qcols[:, col:col + 1], ident_sb[:])
                        q1p_slice = q_1p[:, b * H + c * 128:b * H + (c + 1) * 128]
                        if Q1P_ACT:
                            nc.scalar.copy(q1p_slice, qt_ps[:])
                        else:
                            nc.vector.tensor_copy(q1p_slice, qt_ps[:])

                if mode == "stt":
                    # one cheap approx-reciprocal (no table op) for all four
                    # batches, hidden under the first supertile DMA; ~18
                    # correct bits vs the ~5e-5 f32r rounding already present
                    r_1p = const_pool.tile([1, B_LOC * H], F32, tag="r1p")
                    nc.vector.reciprocal_approx_fast(r_1p[:], q_1p[:])

                # ---- main loop ----------------------------------------------
                trash_a = None
                if mode != "stt" or STT_ACT_TILES > 0:
                    trash_a = const_pool.tile([128, H], F32, tag="trash_a")

                for b in range(B_LOC):
                    qb = qb_pool.tile([128, H], F32, tag="qb")
                    nc.gpsimd.partition_broadcast(
                        qb[:], q_1p[:, b * H:(b + 1) * H])
                    qb3 = qb[:].unsqueeze(1).broadcast_to((128, G, H))

                    if mode in ("br", "brx"):
                        recip_q = qb_pool.tile([1, H], F32, tag="rq")
                        nc.vector.reciprocal(
                            recip_q[:], q_1p[:, b * H:(b + 1) * H])


                    if mode == "a32":
                        # one accumulator column per 2KB PSUM zero region so the
                        # four interleaved accumulation groups don't clobber
                        # each other's pending-zero state
                        ctx_ps = psum_acc_pool.tile([128, 4 * 512], F32, tag="ctx")
                    else:
                        ctx_ps = psum_pool.tile([1, H], F32, tag="ctx")

                    # segments of whole s-tiles; the very last one is split
                    # in half so the end-of-kernel drain chain is shorter
                    segs = [(st * G, G) for st in range(N_ST)]
                    # split only on the truly-final rep: the short segments
                    # shorten the end-of-NEFF drain, but their 4KB packets
                    # stream slower — a net loss at interior rep boundaries
                    if b == B_LOC - 1 and _rep == reps - 1:
                        g0, gn = segs.pop()
                        q = max(1, gn // 4)
                        segs += [(g, q) for g in range(g0, g0 + gn, q)]
                    for si, (g0, gn) in enumerate(segs):
                        first = si == 0
                        last = si == len(segs) - 1
                        stile = st_pool.tile([128, gn * H], F32, tag="st")
                        # s<->partition mapping differs between the two forms
                        # but scores/tmp/matmul all use the same relabeling,
                        # so the contraction is unchanged.
                        src = enc[b, g0 * 128:(g0 + gn) * 128, :]
                        src = (src.rearrange("(p g) h -> p g h", g=gn)
                               if DMA_PMAJOR else
                               src.rearrange("(g p) h -> p g h", p=128))
                        if DMA_GP_NTH and (b * N_ST + si) % DMA_GP_NTH == DMA_GP_NTH - 1:
                            dma_eng = nc.gpsimd
                        elif DMA_ALT and si % 2:
                            dma_eng = nc.scalar
                        else:
                            dma_eng = nc.sync
                        dma_eng.dma_start(
                            stile[:].rearrange("p (g h) -> p g h", g=gn), src)
                        st3 = stile[:].rearrange("p (g h) -> p g h", g=gn)

                        if mode == "stt":
                            # fused mul+reduce: ONE DVE pass per 512-wide
                            # tile writes tmp (f32r, feeds the matmul) and
                            # the scores column via accum_out — replaces the
                            # baseline's separate mul + reduce passes
                            tmp = tmp_pool.tile([128, gn * H], F32R, tag="tmp")
                            tmp3 = tmp[:].rearrange("p (g h) -> p g h", g=gn)
                            scores = sc_pool.tile([128, gn], F32R, tag="sc")
                            with nc.allow_low_precision(
                                    reason="f32r tmp/scores feed f32r matmul"):
                                n_act = min(STT_ACT_TILES, gn - 1)
                                for j in range(gn - n_act):
                                    # fused mul+reduce: one DVE pass writes
                                    # tmp and the scores column
                                    nc.vector.scalar_tensor_tensor(
                                        out=tmp3[:, j, :],
                                        in0=st3[:, j, :],
                                        scalar=1.0,
                                        in1=qb[:],
                                        op0=mybir.AluOpType.mult,
                                        op1=mybir.AluOpType.mult,
                                        accum_out=scores[:, j:j + 1],
                                    )
                                if n_act:
                                    # remaining tiles: ONE batched DVE mul
                                    # (cheaper per tile than STT), reduced
                                    # on the otherwise-idle ACT engine
                                    j0 = gn - n_act
                                    nc.vector.tensor_mul(
                                        tmp3[:, j0:gn, :], st3[:, j0:gn, :],
                                        qb3[:, j0:gn, :])
                                    for j in range(j0, gn):
                                        nc.scalar.activation(
                                            trash_a[:],
                                            tmp3[:, j, :].bitcast(F32),
                                            mybir.ActivationFunctionType.Copy,
                                            accum_out=scores[:, j:j + 1],
                                        )
                            for j in range(gn):
                                nc.tensor.matmul(
                                    ctx_ps[:], scores[:, j:j + 1],
                                    tmp[:, j * H:(j + 1) * H],
                                    start=(first and j == 0),
                                    stop=(last and j == gn - 1),
                                )
                            continue

                        is_r = mode in ("br", "brx")
                        tmp_dt = F32R if is_r else F32
                        tmp = tmp_pool.tile([128, gn * H], tmp_dt, tag="tmp")
                        tmp3 = tmp[:].rearrange("p (g h) -> p g h", g=gn)
                        if "dve" not in SKIP:
                            nc.vector.tensor_mul(tmp3, st3, qb3[:, :gn, :])
                        tmp3f = tmp3.bitcast(F32) if is_r else tmp3

                        sc_dt = F32R if is_r else F32
                        scores = sc_pool.tile([128, gn], sc_dt, tag="sc")
                        n_dve = min(BRX_DVE_TILES, gn - 1) if mode == "brx" else 0
                        with nc.allow_low_precision(
                                reason="f32r scores feed f32r matmul"):
                            if "act" not in SKIP:
                                for j in range(gn - n_dve):
                                    nc.scalar.activation(
                                        trash_a[:], tmp3f[:, j, :],
                                        mybir.ActivationFunctionType.Copy,
                                        accum_out=scores[:, j:j + 1],
                                    )
                            if n_dve:
                                nc.vector.reduce_sum(
                                    scores[:, gn - n_dve:gn],
                                    tmp3f[:, gn - n_dve:gn, :],
                                    axis=mybir.AxisListType.X,
                                )

                        if mode == "b32":
                            for j in range(gn):
                                nc.tensor.matmul(
                                    ctx_ps[:], scores[:, j:j + 1],
                                    stile[:, j * H:(j + 1) * H],
                                    start=(first and j == 0),
                                    stop=(last and j == gn - 1),
                                )
                        elif mode in ("br", "brx"):
                            for j in range(gn):
                                if "pe" in SKIP and not (
                                        (first and j == 0)
                                        or (last and j == gn - 1)):
                                    continue
                                nc.tensor.matmul(
                                    ctx_ps[:], scores[:, j:j + 1],
                                    tmp[:, j * H:(j + 1) * H],
                                    start=(first and j == 0),
                                    stop=(last and j == gn - 1),
                                )
                        else:  # a32
                            for j in range(gn):
                                for c in range(4):
                                    nc.tensor.matmul(
                                        ctx_ps[:, c * 512:c * 512 + 1],
                                        stile[:, j * H + c * 128:j * H + (c + 1) * 128],
                                        scores[:, j:j + 1],
                                        start=(first and j == 0),
                                        stop=(last and j == gn - 1),
                                    )

                    # ---- batch epilogue -------------------------------------
                    if mode == "a32":
                        ctx4_sb = out_pool.tile([128, 4], F32, tag="c4sb")
                        ctx_cols = ctx_ps[:].rearrange(
                            "p (c z) -> p c z", c=4)[:, :, 0:1].squeeze(2)
                        nc.vector.tensor_copy(ctx4_sb[:], ctx_cols)
                        ct_ps = psum_q_pool.tile([4, 128], F32, tag="ct")
                        nc.tensor.transpose(ct_ps[:], ctx4_sb[:], ident_sb[:])
                        o_sb = out_pool.tile([4, 128], F32, tag="osb")
                        nc.vector.tensor_copy(o_sb[:], ct_ps[:])
                        nc.gpsimd.dma_start(
                            out[b, :].rearrange("(c h) -> c h", c=4), o_sb[:])
                    else:
                        o_sb = out_pool.tile([1, H], F32, tag="osb")
                        tail = b == B_LOC - 1
                        if mode == "stt" and not tail:
                            # PSUM read on ACT (gpsimd can't touch PSUM),
                            # then the tiny elementwise mul on idle Pool
                            o_raw = out_pool.tile([1, H], F32, tag="oraw")
                            nc.scalar.copy(o_raw[:], ctx_ps[:])
                            nc.gpsimd.tensor_mul(
                                o_sb[:], o_raw[:],
                                r_1p[:, b * H:(b + 1) * H])
                        elif mode == "stt":
                            nc.vector.tensor_mul(
                                o_sb[:], ctx_ps[:],
                                r_1p[:, b * H:(b + 1) * H])
                        elif mode in ("br", "brx") and not tail:
                            # PSUM read on ACT (gpsimd can't touch PSUM),
                            # then the tiny elementwise divide on idle Pool
                            o_raw = out_pool.tile([1, H], F32, tag="oraw")
                            nc.scalar.copy(o_raw[:], ctx_ps[:])
                            nc.gpsimd.tensor_mul(o_sb[:], o_raw[:], recip_q[:])
                        elif mode in ("br", "brx"):
                            # last batch: shortest-latency epilogue — one DVE
                            # op straight from PSUM; nothing competes by now
                            nc.vector.tensor_mul(o_sb[:], ctx_ps[:], recip_q[:])
                        else:
                            nc.vector.tensor_copy(o_sb[:], ctx_ps[:])
                        # SWDGE (Pool) so this tiny store never blocks the
                        # SP HWDGE ring that streams the next enc tiles;
                        # the final store takes the now-idle SP ring instead
                        _odma = (nc.sync if tail else
                                 (nc.gpsimd if OUT_ENG == "gp" else nc.sync))
                        _odma.dma_start(out[b:b + 1, :], o_sb[:])

    nc.compile()
    return nc


_NC_CACHE: dict[str, bass.Bass] = {}


def _get_nc(mode: str = MODE) -> bass.Bass:
    if mode not in _NC_CACHE:
        _NC_CACHE[mode] = build_nc(mode)
    return _NC_CACHE[mode]


def make_in_maps(enc_output, dec_output, W):
    enc_output = np.asarray(enc_output, dtype=np.float32)
    dec_output = np.asarray(dec_output, dtype=np.float32)
    W = np.asarray(W, dtype=np.float32)
    ident = np.eye(128, dtype=np.float32)
    in_maps = []
    for c in range(N_CORES):
        sl = slice(c * B_LOC, (c + 1) * B_LOC)
        in_maps.append({
            "enc": np.ascontiguousarray(enc_output[sl]),
            "dec": np.ascontiguousarray(dec_output[sl, 0, :]),
            "w": W,
            "ident": ident,
        })
    return in_maps


def kernel(enc_output, dec_output, W, **run_kwargs):
    nc = _get_nc()
    in_maps = make_in_maps(enc_output, dec_output, W)
    res = run_bass_kernel_spmd(nc, in_maps, core_ids=list(range(N_CORES)),
                               **run_kwargs)
    outs = [res.results[c]["out"] for c in range(N_CORES)]
    full = np.concatenate(outs, axis=0)[:, None, :].astype(np.float32)
    kernel.last_results = res
    return full

